# revision 1
# baseline (speedup 1.0000x reference)
"""Trainium2 Bass kernel for nn_AbstractAtt (MLB-style attention + fusion + classifier).

Data-parallel over 8 NeuronCores: batch 128 -> 16 samples/core, weights replicated.

Per-core pipeline (matmuls in fp32r = TF32-class; Wf/Wc streamed as bf16):
  x_v   = tanh(v^T @ Wv + bv)            [DA, S] orientation, PE + ACT(bias fused)
  x_att = tanh(x_v * x_q[b])             ACT with per-partition scale (x_q^T col)
  scores= x_att^T @ Wa                   [G, S] PSUM accumulation over DA tiles
  att   = exp(scores + ba); row-sums via ACT accum_out; normalization folded
          into the pooled output (softmax denominator applied post-pooling)
  pool  = e^T @ v^T                      needs v^T: PE-transposed per sample
  xv    = tanh(v_att @ Wf[g] + bf)       glimpse-packed lhsT (vaT columns g*16+b)
  x     = tanh(xv * xq);  out = x @ Wc + bc

DMA-count discipline: the HWDGE ring costs ~0.6us serial per dma_start, so
weight/input streams are merged into multi-k-tile transfers.
"""

import os

import ml_dtypes
import numpy as np

import concourse.bass as bass
import concourse.mybir as mybir
import concourse.tile as tile
from concourse import bacc
from concourse.bass_utils import run_bass_kernel_spmd
from concourse.masks import make_identity

F32 = mybir.dt.float32
F32R = mybir.dt.float32r
BF16 = mybir.dt.bfloat16
AF = mybir.ActivationFunctionType
KVARIANT = os.environ.get("KVARIANT", "full")  # timing probes: full|pairs|notrans
NPAIR_OVR = int(os.environ.get("NPAIRS", "0")) or None

# problem constants (hardcoded per contract)
B, DV, W, H = 128, 2048, 14, 14
S = W * H            # 196
DQ = 2048
DA = 1200
G = 4
DH = 2048
DHG = DH // G        # 512
NANS = 3000
NCORES = 8
BPC = B // NCORES    # 16 samples per core
NPAIR = BPC // 2     # 8 pairs

NK = DV // 128       # 16 k-tiles over DV (== DQ // 128)
KM = 4               # k-tiles merged per DMA / per SBUF tile
NKM = NK // KM       # 4 merged groups
DA_TILES = [(m * 128, min(128, DA - m * 128)) for m in range((DA + 127) // 128)]
S0, S1 = 128, S - 128          # S split 128 + 68
S2 = 2 * S                     # 392 columns per pair
NANS_TILES = [(j * 500, 500) for j in range(6)]
XQF_TILES = [(j * 256, 256) for j in range(8)]


def build_module(reps: int = 1) -> bacc.Bacc:
    nc = bacc.Bacc("TRN2", target_bir_lowering=False, debug=False)

    v = nc.dram_tensor("v", [BPC, DV, S], F32, kind="ExternalInput").ap()
    q = nc.dram_tensor("q", [BPC, DQ], F32, kind="ExternalInput").ap()
    wv = nc.dram_tensor("wv", [DV, DA], F32, kind="ExternalInput").ap()
    bv = nc.dram_tensor("bv", [DA, 1], F32, kind="ExternalInput").ap()
    wq = nc.dram_tensor("wq", [DQ, DA], F32, kind="ExternalInput").ap()
    bq = nc.dram_tensor("bq", [DA, 1], F32, kind="ExternalInput").ap()
    wa = nc.dram_tensor("wa", [DA, G], F32, kind="ExternalInput").ap()
    ba = nc.dram_tensor("ba", [G, 1], F32, kind="ExternalInput").ap()
    wf = nc.dram_tensor("wf", [G, DV, DHG], BF16, kind="ExternalInput").ap()
    bf = nc.dram_tensor("bf", [1, DH], F32, kind="ExternalInput").ap()
    wqf = nc.dram_tensor("wqf", [DQ, DH], F32, kind="ExternalInput").ap()
    bqf = nc.dram_tensor("bqf", [1, DH], F32, kind="ExternalInput").ap()
    wc = nc.dram_tensor("wc", [DQ, NANS], BF16, kind="ExternalInput").ap()
    bc = nc.dram_tensor("bc", [1, NANS], F32, kind="ExternalInput").ap()
    out = nc.dram_tensor("out", [BPC, NANS], F32, kind="ExternalOutput").ap()

    with tile.TileContext(nc) as tc:
        if reps > 4:
            # device-side loop for timing: constant code size, R iterations
            with tc.For_i(0, reps, 1):
                emit_core(nc, tc, v, q, wv, bv, wq, bq, wa, ba, wf, bf, wqf,
                          bqf, wc, bc, out)
        else:
            for rep in range(reps):
                emit_core(nc, tc, v, q, wv, bv, wq, bq, wa, ba, wf, bf, wqf,
                          bqf, wc, bc, out)
    nc.compile()
    return nc


def emit_core(nc, tc, v, q, wv, bv, wq, bq, wa, ba, wf, bf, wqf, bqf, wc, bc, out):
    from contextlib import ExitStack

    ctx = ExitStack()
    with ctx:
        # ---------------- persistent pools ----------------
        const_pool = ctx.enter_context(tc.tile_pool(name="const", bufs=1))
        wv_pool = ctx.enter_context(tc.tile_pool(name="wvp", bufs=1))

        ident = const_pool.tile([128, 128], F32)
        make_identity(nc, ident[:])
        ident_r = const_pool.tile([128, 128], F32R)
        nc.vector.tensor_copy(ident_r[:], ident[:])
        ones_f = const_pool.tile([1, 16], F32)
        nc.gpsimd.memset(ones_f[:], 1.0)
        ones = const_pool.tile([1, 16], F32R)
        nc.vector.tensor_copy(ones[:], ones_f[:])

        # per-partition bias tiles, packed loads (col m = DA tile m)
        bv_sb = const_pool.tile([128, len(DA_TILES)], F32)
        bq_sb = const_pool.tile([128, len(DA_TILES)], F32)
        nc.sync.dma_start(bv_sb[:, :9], bv[0:1152, 0].rearrange("(m p) -> p m", p=128))
        nc.sync.dma_start(bv_sb[:48, 9:10], bv[1152:1200, :])
        nc.sync.dma_start(bq_sb[:, :9], bq[0:1152, 0].rearrange("(m p) -> p m", p=128))
        nc.sync.dma_start(bq_sb[:48, 9:10], bq[1152:1200, :])
        ba_sb = const_pool.tile([G, 1], F32)
        nc.sync.dma_start(ba_sb[:], ba[:])
        # wa packed: [128, 40] f32r, cols m*4..m*4+4 = Wa rows m*128..+128
        wa_sb = const_pool.tile([128, G * len(DA_TILES)], F32R)
        nc.sync.dma_start(
            wa_sb[:, :36].rearrange("p (m g) -> p m g", g=G),
            wa[0:1152, :].rearrange("(m p) g -> p m g", p=128).bitcast(F32R))
        nc.sync.dma_start(wa_sb[:48, 36:40], wa[1152:1200, :].bitcast(F32R))
        bqf_sb = const_pool.tile([1, DH], F32R)
        nc.sync.dma_start(bqf_sb[:], bqf[:].bitcast(F32R))

        # ---------------- pre-phase: q^T, x_q^T ----------------
        with tc.tile_pool(name="pre", bufs=1) as pre, \
             tc.tile_pool(name="pre_ps", bufs=1, space="PSUM") as pre_ps:
            q_sb = pre.tile([BPC, DQ], F32)
            nc.sync.dma_start(q_sb[:], q[:])
            # qT: [DQ(k-tiles), BPC] in f32r, one tile [128, NK*16]
            qT = const_pool.tile([128, NK * BPC], F32R)
            for k in range(NK):
                p = pre_ps.tile([128, BPC], F32, tag="qt", bufs=2)
                nc.tensor.transpose(p[:], q_sb[:, k * 128:(k + 1) * 128],
                                    ident[:BPC, :BPC])
                nc.vector.tensor_copy(qT[:, k * BPC:(k + 1) * BPC], p[:])

            # x_q_lin = q @ Wq  ([BPC, DA]); Wq streamed 4-k-merged per chunk
            xq_lin = pre.tile([BPC, DA], F32)
            for j, (n0, nw) in enumerate([(0, 400), (400, 400), (800, 400)]):
                pj = pre_ps.tile([BPC, nw], F32, tag=f"xq{j}")
                for kk in range(NKM):
                    wt = pre.tile([128, KM * nw], F32R, tag="wqs", bufs=2)
                    nc.sync.dma_start(
                        wt[:].rearrange("p (k n) -> p k n", k=KM),
                        wq[kk * KM * 128:(kk + 1) * KM * 128, n0:n0 + nw]
                        .rearrange("(k c) n -> c k n", k=KM).bitcast(F32R))
                    for ki in range(KM):
                        k = kk * KM + ki
                        nc.tensor.matmul(pj[:], qT[:, k * BPC:(k + 1) * BPC],
                                         wt[:, ki * nw:(ki + 1) * nw],
                                         start=(k == 0), stop=(k == NK - 1))
                nc.vector.tensor_copy(xq_lin[:, n0:n0 + nw], pj[:])

            # x_qT[m] = tanh(xq_lin^T + bq) per DA tile  -> [mw, BPC] f32
            xqT = const_pool.tile([128, len(DA_TILES) * BPC], F32)
            for m, (m0, mw) in enumerate(DA_TILES):
                p = pre_ps.tile([128, BPC], F32, tag="qt", bufs=2)
                nc.tensor.transpose(p[:mw, :], xq_lin[:, m0:m0 + mw],
                                    ident[:BPC, :BPC])
                nc.scalar.activation(xqT[:mw, m * BPC:(m + 1) * BPC], p[:mw, :],
                                     AF.Tanh, bias=bq_sb[:mw, m:m + 1])

        # resident Wv tiles (f32r), 4-k-merged: wv_sb[kk][:, ki*DA + d]
        wv_sb = []
        for kk in range(NKM):
            t = wv_pool.tile([128, KM * DA], F32R, tag=f"wv{kk}")
            nc.sync.dma_start(
                t[:].rearrange("p (k d) -> p k d", k=KM),
                wv[kk * KM * 128:(kk + 1) * KM * 128, :]
                .rearrange("(k c) d -> c k d", k=KM).bitcast(F32R))
            wv_sb.append(t)

        def wv_lhsT(k, m0, mw):
            return wv_sb[k // KM][:, (k % KM) * DA + m0:(k % KM) * DA + m0 + mw]

        # xqf accumulator [BPC, DH], built in chunks interleaved with pair loop
        xqf_sb = const_pool.tile([BPC, DH], F32)
        # v_att collection [4*BPC, DV] (partition = 4*b + g)
        vatt_sb = const_pool.tile([4 * BPC, DV], F32)

        # ---------------- pair loop ----------------
        # wf stream pool hoisted so its slots exist early -> prefetch in pairs
        wfp = ctx.enter_context(tc.tile_pool(name="wfp", bufs=1))
        with tc.tile_pool(name="pl", bufs=1) as pl, \
             tc.tile_pool(name="pl_ps", bufs=1, space="PSUM") as pl_ps:

            def xqf_chunk(j):
                n0, nw = XQF_TILES[j]
                pj = pl_ps.tile([BPC, nw], F32, tag="pqf", bufs=1)
                for kk in range(NKM):
                    wt = pl.tile([128, KM * nw], F32R, tag="wqf", bufs=3)
                    nc.sync.dma_start(
                        wt[:].rearrange("p (k n) -> p k n", k=KM),
                        wqf[kk * KM * 128:(kk + 1) * KM * 128, n0:n0 + nw]
                        .rearrange("(k c) n -> c k n", k=KM).bitcast(F32R))
                    for ki in range(KM):
                        k = kk * KM + ki
                        nc.tensor.matmul(pj[:], qT[:, k * BPC:(k + 1) * BPC],
                                         wt[:, ki * nw:(ki + 1) * nw],
                                         start=(k == 0), stop=False,
                                         skip_group_check=True)
                nc.tensor.matmul(pj[:], ones[:, :BPC], bqf_sb[:, n0:n0 + nw],
                                 start=False, stop=True, skip_group_check=True)
                nc.scalar.activation(xqf_sb[:, n0:n0 + nw], pj[:], AF.Tanh)

            for pair in range(NPAIR_OVR or NPAIR):
                b0 = pair * 2
                # v pair tiles, 4-k-merged: v_sb[kk][:, (ki, b, s)]
                v_sb = []
                for kk in range(NKM):
                    t = pl.tile([128, KM * S2], F32R, tag=f"v{kk}", bufs=2)
                    tv = t[:].rearrange("p (k b s) -> p k b s", k=KM, b=2)
                    for s in range(2):
                        nc.sync.dma_start(
                            tv[:, :, s, :],
                            v[b0 + s, kk * KM * 128:(kk + 1) * KM * 128, :]
                            .rearrange("(k c) s -> c k s", k=KM).bitcast(F32R))
                    v_sb.append(t)

                def v_rhs(k, lo=0, width=S2):
                    return v_sb[k // KM][:, (k % KM) * S2 + lo:
                                         (k % KM) * S2 + lo + width]

                p_sc = pl_ps.tile([G, S2], F32, tag="psc", bufs=1)
                for m, (m0, mw) in enumerate(DA_TILES):
                    pm = pl_ps.tile([128, S2], F32, tag="pmain", bufs=2)
                    for k in range(NK):
                        nc.tensor.matmul(pm[:mw, :], wv_lhsT(k, m0, mw),
                                         v_rhs(k), start=(k == 0),
                                         stop=(k == NK - 1),
                                         skip_group_check=True)
                    # x_v = tanh(mm + bv)
                    xv_t = pl.tile([128, S2], F32, tag="xv", bufs=2)
                    nc.scalar.activation(xv_t[:mw, :], pm[:mw, :], AF.Tanh,
                                         bias=bv_sb[:mw, m:m + 1])
                    # x_att = tanh(x_v * xq[b])  per sample
                    xa_t = pl.tile([128, S2], F32R, tag="xa", bufs=2)
                    for s in range(2):
                        nc.scalar.activation(
                            xa_t[:mw, s * S:(s + 1) * S],
                            xv_t[:mw, s * S:(s + 1) * S], AF.Tanh,
                            scale=xqT[:mw, m * BPC + b0 + s:m * BPC + b0 + s + 1])
                    # scores accumulation
                    nc.tensor.matmul(p_sc[:], wa_sb[:mw, m * G:(m + 1) * G],
                                     xa_t[:mw, :],
                                     start=(m == 0), stop=(m == len(DA_TILES) - 1),
                                     skip_group_check=True)

                # att = exp(scores + ba), with per-sample row sums
                e_sb = pl.tile([G, S2], F32, tag="e", bufs=1)
                esum = pl.tile([G, 2], F32, tag="esum", bufs=2)
                for s in range(2):
                    nc.scalar.activation(e_sb[:, s * S:(s + 1) * S],
                                         p_sc[:, s * S:(s + 1) * S], AF.Exp,
                                         bias=ba_sb[:], accum_out=esum[:, s:s + 1])
                recip = pl.tile([G, 2], F32, tag="recip", bufs=2)
                nc.vector.reciprocal(recip[:], esum[:])

                for s in range(2):
                    # e^T tiles for this sample: [128,4] + [68,4]
                    eT0 = pl.tile([S0, G], F32R, tag="eT0", bufs=2)
                    eT1 = pl.tile([S1, G], F32R, tag="eT1", bufs=2)
                    pt0 = pl_ps.tile([S0, G], F32, tag="peT", bufs=1)
                    nc.tensor.transpose(pt0[:], e_sb[:, s * S:s * S + S0],
                                        ident[:G, :G])
                    nc.vector.tensor_copy(eT0[:], pt0[:])
                    pt1 = pl_ps.tile([S0, G], F32, tag="peT", bufs=1)
                    nc.tensor.transpose(pt1[:S1, :], e_sb[:, s * S + S0:(s + 1) * S],
                                        ident[:G, :G])
                    nc.vector.tensor_copy(eT1[:], pt1[:S1, :])

                    # v^T tiles for this sample: [128, DV] and [68, DV]
                    if KVARIANT == "notrans":
                        vT0 = vT1 = None
                    else:
                        vT0 = pl.tile([S0, DV], F32R, tag="vT0", bufs=1)
                    if KVARIANT != "notrans":
                        vT1 = pl.tile([S1, DV], F32R, tag="vT1", bufs=1)
                    for k in range(NK if KVARIANT != "notrans" else 0):
                        pv0 = pl_ps.tile([S0, 128], F32R, tag="pvT", bufs=2)
                        nc.tensor.transpose(pv0[:], v_rhs(k, s * S, S0), ident_r[:])
                        nc.vector.tensor_copy(
                            vT0[:, k * 128:(k + 1) * 128], pv0[:].bitcast(F32))
                        pv1 = pl_ps.tile([S0, 128], F32R, tag="pvT", bufs=2)
                        nc.tensor.transpose(pv1[:S1, :], v_rhs(k, s * S + S0, S1),
                                            ident_r[:])
                        nc.vector.tensor_copy(
                            vT1[:, k * 128:(k + 1) * 128], pv1[:S1, :].bitcast(F32))

                    # pooling: U[g, c] = e^T @ v^T; normalize into tmp; 1 DMA
                    tmp = pl.tile([G, DV], F32, tag="ptmp", bufs=1)
                    for cchunk in range(DV // 512):
                        c0 = cchunk * 512
                        pp = pl_ps.tile([G, 512], F32, tag="ppool", bufs=1)
                        if KVARIANT == "notrans":
                            nc.tensor.matmul(pp[:], eT0[:], v_sb[cchunk][:, :512],
                                             start=True, stop=False)
                            nc.tensor.matmul(pp[:], eT1[:], v_sb[cchunk][:68, :512],
                                             start=False, stop=True)
                        else:
                            nc.tensor.matmul(pp[:], eT0[:], vT0[:, c0:c0 + 512],
                                             start=True, stop=False)
                            nc.tensor.matmul(pp[:], eT1[:], vT1[:, c0:c0 + 512],
                                             start=False, stop=True)
                        nc.vector.tensor_scalar_mul(tmp[:, c0:c0 + 512], pp[:],
                                                    recip[:, s:s + 1])
                    nc.sync.dma_start(
                        vatt_sb[(b0 + s) * G:(b0 + s + 1) * G, :], tmp[:])

                xqf_chunk(pair)

        if KVARIANT in ("pairs", "notrans"):
            nc.sync.dma_start(out[:, :DH], vatt_sb[:BPC, :])
            return
        # ---------------- tail: vaT transpose, fused fusion+classifier ----------
        with tc.tile_pool(name="tl", bufs=1) as tl:
            bf_sb = tl.tile([1, DH], F32R)
            nc.sync.dma_start(bf_sb[:], bf[:].bitcast(F32R))
            bc_sb = tl.tile([1, NANS], F32R)
            nc.sync.dma_start(bc_sb[:], bc[:].bitcast(F32R))

            # vaT[k]: [128, 64] bf16, columns g*16+b (own psum scope)
            vaT = []
            with tc.tile_pool(name="vat_ps", bufs=1, space="PSUM") as vat_ps:
                for k in range(NK):
                    t = tl.tile([128, G * BPC], BF16, tag=f"vaT{k}")
                    p = vat_ps.tile([128, G * BPC], F32, tag="pvat", bufs=2)
                    nc.tensor.transpose(p[:], vatt_sb[:, k * 128:(k + 1) * 128],
                                        ident[:G * BPC, :G * BPC])
                    for g in range(G):
                        nc.vector.tensor_copy(
                            t[:, g * BPC:(g + 1) * BPC],
                            p[:, g:G * BPC:G])
                    vaT.append(t)

            # staged: D (all glimpses) -> E (all k) -> classifier (j-outer)
            with tc.tile_pool(name="tl_ps", bufs=1, space="PSUM") as tl_ps:
                xv_sb = tl.tile([BPC, DH], F32)
                out_sb = tl.tile([BPC, NANS], F32)
                for g in range(G):
                    pd = tl_ps.tile([BPC, DHG], F32, tag="pd", bufs=2)
                    wts = []
                    for kk in range(NKM):
                        wt = wfp.tile([128, KM * DHG], BF16, tag="wfs", bufs=3)
                        nc.sync.dma_start(
                            wt[:].rearrange("p (k n) -> p k n", k=KM),
                            wf[g, kk * KM * 128:(kk + 1) * KM * 128, :]
                            .rearrange("(k c) n -> c k n", k=KM))
                        wts.append(wt)
                    for k in range(NK):
                        nc.tensor.matmul(
                            pd[:], vaT[k][:, g * BPC:(g + 1) * BPC],
                            wts[k // KM][:, (k % KM) * DHG:(k % KM + 1) * DHG],
                            start=(k == 0), stop=False, skip_group_check=True)
                    nc.tensor.matmul(pd[:], ones[:, :BPC],
                                     bf_sb[:, g * DHG:(g + 1) * DHG],
                                     start=False, stop=True,
                                     skip_group_check=True)
                    nc.scalar.activation(xv_sb[:, g * DHG:(g + 1) * DHG], pd[:],
                                         AF.Tanh)
                # E: x = tanh(xv * xqf) transposed into xT[k] tiles (bf16)
                xT = []
                for k in range(NK):
                    xmk = tl.tile([BPC, 128], F32, tag="xmk", bufs=3)
                    nc.vector.tensor_mul(xmk[:], xv_sb[:, k * 128:(k + 1) * 128],
                                         xqf_sb[:, k * 128:(k + 1) * 128])
                    px = tl_ps.tile([128, BPC], F32, tag="pxT", bufs=3)
                    nc.tensor.transpose(px[:], xmk[:], ident[:BPC, :BPC])
                    xTk = tl.tile([128, BPC], BF16, tag=f"xT{k}")
                    nc.scalar.activation(xTk[:], px[:], AF.Tanh)
                    xT.append(xTk)
                # classifier j-outer, Wc streamed 4-k-merged per (j, kk)
                for j, (n0, nw) in enumerate(NANS_TILES):
                    pc = tl_ps.tile([BPC, nw], F32, tag="pc", bufs=2)
                    for kk in range(NKM):
                        wct = tl.tile([128, KM * nw], BF16, tag="wcs", bufs=6)
                        nc.sync.dma_start(
                            wct[:].rearrange("p (k n) -> p k n", k=KM),
                            wc[kk * KM * 128:(kk + 1) * KM * 128, n0:n0 + nw]
                            .rearrange("(k c) n -> c k n", k=KM))
                        for ki in range(KM):
                            k = kk * KM + ki
                            nc.tensor.matmul(pc[:], xT[k][:],
                                             wct[:, ki * nw:(ki + 1) * nw],
                                             start=(k == 0), stop=False,
                                             skip_group_check=True)
                    nc.tensor.matmul(pc[:], ones[:, :BPC], bc_sb[:, n0:n0 + nw],
                                     start=False, stop=True,
                                     skip_group_check=True)
                    nc.vector.tensor_copy(out_sb[:, n0:n0 + nw], pc[:])
                nc.sync.dma_start(out[:], out_sb[:])


_module_cache = {}


def _get_module(reps: int = 1):
    if reps not in _module_cache:
        _module_cache[reps] = build_module(reps)
    return _module_cache[reps]


def make_in_maps(inputs: dict) -> list:
    iv = np.ascontiguousarray(inputs["input_v"], np.float32).reshape(B, DV, S)
    xq = np.ascontiguousarray(inputs["x_q_vec"], np.float32)
    shared = {
        "wv": np.ascontiguousarray(inputs["Wv_att"], np.float32),
        "bv": np.ascontiguousarray(inputs["bv_att"], np.float32).reshape(DA, 1),
        "wq": np.ascontiguousarray(inputs["Wq_att"], np.float32),
        "bq": np.ascontiguousarray(inputs["bq_att"], np.float32).reshape(DA, 1),
        "wa": np.ascontiguousarray(inputs["Wa"], np.float32),
        "ba": np.ascontiguousarray(inputs["ba"], np.float32).reshape(G, 1),
        "wf": np.ascontiguousarray(inputs["Wf"]).astype(ml_dtypes.bfloat16),
        "bf": np.ascontiguousarray(inputs["bf"], np.float32).reshape(1, DH),
        "wqf": np.ascontiguousarray(inputs["Wqf"], np.float32),
        "bqf": np.ascontiguousarray(inputs["bqf"], np.float32).reshape(1, DH),
        "wc": np.ascontiguousarray(inputs["Wc"]).astype(ml_dtypes.bfloat16),
        "bc": np.ascontiguousarray(inputs["bc"], np.float32).reshape(1, NANS),
    }
    in_maps = []
    for c in range(NCORES):
        m = dict(shared)
        m["v"] = np.ascontiguousarray(iv[c * BPC:(c + 1) * BPC])
        m["q"] = np.ascontiguousarray(xq[c * BPC:(c + 1) * BPC])
        in_maps.append(m)
    return in_maps


def kernel(**inputs) -> np.ndarray:
    nc = _get_module(1)
    in_maps = make_in_maps(inputs)
    res = run_bass_kernel_spmd(nc, in_maps, core_ids=list(range(NCORES)))
    return np.concatenate([res.results[c]["out"] for c in range(NCORES)], axis=0)



# revision 12
# speedup vs baseline: 1005.4139x; 1005.4139x over previous
"""Trainium2 Bass kernel for nn_AbstractAtt (MLB-style attention + fusion + classifier).

Data-parallel over 8 NeuronCores: batch 128 -> 16 samples/core, weights replicated.

v2 design (vs fp32r v1):
  - main x_v matmul in fp8(e4m3) DoubleRow perf mode: 2 k-subtiles contracted
    per pass (2x PE throughput). Wv host-prescaled by 32 (fp8 subnormal range),
    rescale folded into the activation's scale. v/Wv host-packed into the
    (k-pair, ...) interleaved layout DoubleRow wants, so DMAs are contiguous.
  - v^T for pooling is host-transposed and DMAed (bf16), killing all PE
    v-transposes and their PSUM->SBUF DVE copies.
  - x_att = tanh(x_v * x_q) via ONE broadcast DVE multiply (bf16, 2x/4x mode)
    + ONE batched ACT tanh per pair, instead of 2 small ACT ops per DA tile.
  - everything else bf16 (q, Wq, Wqf, Wa, Wf, Wc streams, xqf, pooling).
  - e^T built with 4 PE transposes packed into one PSUM bank (start=False
    accumulate onto the zero region) + one DVE copy.
  - Wf fully prefetched during the pair loop; Wc streamed from tail start.

Per-core pipeline:
  x_v   = tanh((v8^T @ Wv8) / 32 + bv)    [DA, 2S] per pair, fp8 DoubleRow
  x_att = tanh(x_v * x_q[b])              DVE bcast-mul + ACT tanh (batched)
  scores= Wa^T @ x_att                    [G, 2S] PSUM accum over DA tiles
  att   = exp(scores + ba); row sums via ACT accum_out; softmax denominator
          applied on the pooled output (DVE scalar mul by 1/sum)
  pool  = e^T @ v^T                       v^T host-provided bf16
  xv    = tanh(v_att @ Wf[g] + bf)        glimpse-strided lhsT columns
  x     = tanh(xv * xq);  out = x @ Wc + bc
"""

import os

import ml_dtypes
import numpy as np

import concourse.bass as bass
import concourse.mybir as mybir
import concourse.tile as tile
from concourse import bacc
from concourse.bass_utils import run_bass_kernel_spmd
from concourse.masks import make_identity

F32 = mybir.dt.float32
BF16 = mybir.dt.bfloat16
F8 = mybir.dt.float8e4
AF = mybir.ActivationFunctionType
DR = mybir.MatmulPerfMode.DoubleRow

# problem constants (hardcoded per contract)
B, DV, W, H = 128, 2048, 14, 14
S = W * H            # 196
DQ = 2048
DA = 1200
G = 4
DH = 2048
DHG = DH // G        # 512
NANS = 3000
NCORES = 8
BPC = B // NCORES    # 16 samples per core
NPAIR = BPC // 2     # 8 pairs

NK = DV // 128       # 16 k-subtiles over DV (== DQ // 128)
KM = 4               # k-subtiles merged per streamed-weight DMA
NKM = NK // KM       # 4 merged groups
KM8 = 4              # k-subtiles per fp8 v/wv SBUF tile (2 DoubleRow pairs)
NKK = NK // KM8      # 4 fp8 tile groups
DA_TILES = [(m * 128, min(128, DA - m * 128)) for m in range((DA + 127) // 128)]
NM = len(DA_TILES)   # 10
S2 = 2 * S           # 392 columns per pair
W8SCALE = 32.0       # host premultiply on Wv before fp8 quant
NANS_TILES = [(j * 500, 500) for j in range(6)]
XQF_TILES = [(j * 256, 256) for j in range(8)]


def build_module(reps: int = 1) -> bacc.Bacc:
    nc = bacc.Bacc("TRN2", target_bir_lowering=False, debug=False)

    # fp8 packed inputs: exact SBUF image per (pair, kk): [128, (ki, b, s)]
    v8 = nc.dram_tensor("v8", [NPAIR, NKK, 128, KM8 * 2 * S], F8,
                        kind="ExternalInput").ap()
    # host-transposed v for pooling: [b, s, c] bf16
    vt = nc.dram_tensor("vt", [BPC, S, DV], BF16, kind="ExternalInput").ap()
    wv8 = nc.dram_tensor("wv8", [NKK, 128, KM8 * DA], F8,
                         kind="ExternalInput").ap()
    q = nc.dram_tensor("q", [BPC, DQ], BF16, kind="ExternalInput").ap()
    wq = nc.dram_tensor("wq", [DQ, DA], BF16, kind="ExternalInput").ap()
    bv = nc.dram_tensor("bv", [DA, 1], F32, kind="ExternalInput").ap()
    bq = nc.dram_tensor("bq", [DA, 1], F32, kind="ExternalInput").ap()
    wa = nc.dram_tensor("wa", [DA, G], BF16, kind="ExternalInput").ap()
    ba = nc.dram_tensor("ba", [G, 1], F32, kind="ExternalInput").ap()
    wf = nc.dram_tensor("wf", [G, DV, DHG], BF16, kind="ExternalInput").ap()
    bfb = nc.dram_tensor("bfb", [1, DH], BF16, kind="ExternalInput").ap()
    wqf = nc.dram_tensor("wqf", [DQ, DH], BF16, kind="ExternalInput").ap()
    bqf = nc.dram_tensor("bqf", [1, DH], BF16, kind="ExternalInput").ap()
    wc = nc.dram_tensor("wc", [DQ, NANS], BF16, kind="ExternalInput").ap()
    bc = nc.dram_tensor("bc", [1, NANS], BF16, kind="ExternalInput").ap()
    out = nc.dram_tensor("out", [BPC, NANS], F32, kind="ExternalOutput").ap()

    args = (v8, vt, wv8, q, wq, bv, bq, wa, ba, wf, bfb, wqf, bqf, wc, bc, out)
    with tile.TileContext(nc) as tc:
        if reps > 4:
            with tc.For_i(0, reps, 1):
                emit_core(nc, tc, *args)
        else:
            for _ in range(reps):
                emit_core(nc, tc, *args)
    nc.compile()
    return nc


def emit_core(nc, tc, v8, vt, wv8, q, wq, bv, bq, wa, ba, wf, bfb, wqf, bqf,
              wc, bc, out):
    from contextlib import ExitStack

    ctx = ExitStack()
    with ctx:
        # ---------------- persistent pools ----------------
        const_pool = ctx.enter_context(tc.tile_pool(name="const", bufs=1))

        ident_f = const_pool.tile([128, 128], F32)
        make_identity(nc, ident_f[:])
        ident = const_pool.tile([128, 128], BF16)
        nc.vector.tensor_copy(ident[:], ident_f[:])
        ones = const_pool.tile([1, BPC], BF16)
        nc.gpsimd.memset(ones[:], 1.0)

        # per-partition bias tiles, packed loads (col m = DA tile m)
        bv_sb = const_pool.tile([128, NM], F32)
        bq_sb = const_pool.tile([128, NM], F32)
        nc.sync.dma_start(bv_sb[:, :9], bv[0:1152, 0].rearrange("(m p) -> p m", p=128))
        nc.sync.dma_start(bv_sb[:48, 9:10], bv[1152:1200, :])
        nc.sync.dma_start(bq_sb[:, :9], bq[0:1152, 0].rearrange("(m p) -> p m", p=128))
        nc.sync.dma_start(bq_sb[:48, 9:10], bq[1152:1200, :])
        ba_sb = const_pool.tile([G, 1], F32)
        nc.sync.dma_start(ba_sb[:], ba[:])
        # wa packed: [128, 40] bf16, cols m*4..m*4+4 = Wa rows m*128..+128
        wa_sb = const_pool.tile([128, G * NM], BF16)
        nc.sync.dma_start(
            wa_sb[:, :36].rearrange("p (m g) -> p m g", g=G),
            wa[0:1152, :].rearrange("(m p) g -> p m g", p=128))
        nc.sync.dma_start(wa_sb[:48, 36:40], wa[1152:1200, :])
        bqf_sb = const_pool.tile([1, DH], BF16)
        nc.sync.dma_start(bqf_sb[:], bqf[:])

        # ---------------- pre-phase: q^T, x_q^T ----------------
        with tc.tile_pool(name="pre", bufs=1) as pre, \
             tc.tile_pool(name="pre_ps", bufs=1, space="PSUM") as pre_ps:
            q_sb = pre.tile([BPC, DQ], BF16)
            nc.sync.dma_start(q_sb[:], q[:])
            # qT: [DQ(k-subtiles), BPC] bf16, one tile [128, NK*16]
            qT = const_pool.tile([128, NK * BPC], BF16)
            for k in range(NK):
                p = pre_ps.tile([128, BPC], BF16, tag="qt", bufs=2)
                nc.tensor.transpose(p[:], q_sb[:, k * 128:(k + 1) * 128],
                                    ident[:BPC, :BPC])
                nc.vector.tensor_copy(qT[:, k * BPC:(k + 1) * BPC], p[:])

            # x_q_lin = q @ Wq  ([BPC, DA]); Wq streamed 4-k-merged per chunk
            xq_lin = pre.tile([BPC, DA], BF16)
            for j, (n0, nw) in enumerate([(0, 400), (400, 400), (800, 400)]):
                pj = pre_ps.tile([BPC, nw], F32, tag=f"xq{j}")
                for kk in range(NKM):
                    wt = pre.tile([128, KM * nw], BF16, tag="wqs", bufs=2)
                    nc.sync.dma_start(
                        wt[:].rearrange("p (k n) -> p k n", k=KM),
                        wq[kk * KM * 128:(kk + 1) * KM * 128, n0:n0 + nw]
                        .rearrange("(k c) n -> c k n", k=KM))
                    for ki in range(KM):
                        k = kk * KM + ki
                        nc.tensor.matmul(pj[:], qT[:, k * BPC:(k + 1) * BPC],
                                         wt[:, ki * nw:(ki + 1) * nw],
                                         start=(k == 0), stop=(k == NK - 1))
                nc.vector.tensor_copy(xq_lin[:, n0:n0 + nw], pj[:])

            # x_qT[m] = tanh(xq_lin^T + bq) per DA tile  -> [mw, BPC] bf16
            xqT = const_pool.tile([128, NM * BPC], BF16)
            for m, (m0, mw) in enumerate(DA_TILES):
                p = pre_ps.tile([128, BPC], BF16, tag="qt", bufs=2)
                nc.tensor.transpose(p[:mw, :], xq_lin[:, m0:m0 + mw],
                                    ident[:BPC, :BPC])
                nc.scalar.activation(xqT[:mw, m * BPC:(m + 1) * BPC], p[:mw, :],
                                     AF.Tanh, bias=bq_sb[:mw, m:m + 1])

        # xqf accumulator [BPC, DH] bf16, chunks interleaved with pair loop
        xqf_sb = const_pool.tile([BPC, DH], BF16)
        # v_att collection [BPC*G, DV] bf16 (partition = 4*b + g)
        vatt_sb = const_pool.tile([G * BPC, DV], BF16)

        # hoisted weight-stream pools so prefetch can start in the pair loop
        wfp = ctx.enter_context(tc.tile_pool(name="wfp", bufs=1))
        wf_tiles = {}

        def load_wf(g, kk):
            t = wfp.tile([128, KM * DHG], BF16, tag=f"wf{g}_{kk}")
            nc.sync.dma_start(
                t[:].rearrange("p (k n) -> p k n", k=KM),
                wf[g, kk * KM * 128:(kk + 1) * KM * 128, :]
                .rearrange("(k c) n -> c k n", k=KM))
            wf_tiles[(g, kk)] = t

        # ---------------- pair loop ----------------
        with tc.tile_pool(name="pl", bufs=1) as pl, \
             tc.tile_pool(name="pl_ps", bufs=1, space="PSUM") as pl_ps:

            # resident fp8 Wv tiles (freed with the pair-loop pool):
            # wv8_sb[kk][:, (ki, d)]
            wv8_sb = []
            for kk in range(NKK):
                t = pl.tile([128, KM8 * DA], F8, tag=f"wv{kk}")
                nc.sync.dma_start(t[:], wv8[kk])
                wv8_sb.append(t)

            def xqf_chunk(j):
                n0, nw = XQF_TILES[j]
                pj = pl_ps.tile([BPC, nw], F32, tag="pqf", bufs=1)
                for kk in range(NKM):
                    wt = pl.tile([128, KM * nw], BF16, tag="wqf", bufs=3)
                    nc.sync.dma_start(
                        wt[:].rearrange("p (k n) -> p k n", k=KM),
                        wqf[kk * KM * 128:(kk + 1) * KM * 128, n0:n0 + nw]
                        .rearrange("(k c) n -> c k n", k=KM))
                    for ki in range(KM):
                        k = kk * KM + ki
                        nc.tensor.matmul(pj[:], qT[:, k * BPC:(k + 1) * BPC],
                                         wt[:, ki * nw:(ki + 1) * nw],
                                         start=(k == 0), stop=False,
                                         skip_group_check=True)
                nc.tensor.matmul(pj[:], ones[:], bqf_sb[:, n0:n0 + nw],
                                 start=False, stop=True, skip_group_check=True)
                nc.scalar.activation(xqf_sb[:, n0:n0 + nw], pj[:], AF.Tanh)

            for pair in range(NPAIR):
                b0 = pair * 2
                # fp8 v tiles: [128, (ki, b, s)] per kk, single contiguous DMA
                v8t = []
                for kk in range(NKK):
                    t = pl.tile([128, KM8, 2, S], F8, tag=f"v{kk}", bufs=2)
                    nc.sync.dma_start(
                        t[:].rearrange("p a b s -> p (a b s)"), v8[pair, kk])
                    v8t.append(t)
                # v^T tiles for pooling (bf16, host-transposed)
                vt0, vt1 = [], []
                for s in range(2):
                    t0 = pl.tile([128, DV], BF16, tag=f"vt0{s}", bufs=2)
                    nc.scalar.dma_start(t0[:], vt[b0 + s, 0:128, :])
                    vt0.append(t0)
                    t1 = pl.tile([68, DV], BF16, tag=f"vt1{s}", bufs=2)
                    nc.scalar.dma_start(t1[:], vt[b0 + s, 128:S, :])
                    vt1.append(t1)

                # x_v = tanh(mm/W8SCALE + bv), DoubleRow fp8 accumulation
                xv_all = pl.tile([128, NM * S2], BF16, tag="xv", bufs=2)
                for m, (m0, mw) in enumerate(DA_TILES):
                    pm = pl_ps.tile([128, S2], F32, tag="pmain", bufs=2)
                    for kk in range(NKK):
                        wr = wv8_sb[kk][:].rearrange("p (k d) -> p k d", k=KM8)
                        for t2 in range(KM8 // 2):
                            nc.tensor.matmul(
                                pm[:mw, :],
                                wr[:, 2 * t2:2 * t2 + 2, m0:m0 + mw],
                                v8t[kk][:, 2 * t2:2 * t2 + 2, :, :],
                                start=(kk == 0 and t2 == 0),
                                stop=(kk == NKK - 1 and t2 == KM8 // 2 - 1),
                                perf_mode=DR, skip_group_check=True)
                    nc.scalar.activation(xv_all[:mw, m * S2:(m + 1) * S2],
                                         pm[:mw, :], AF.Tanh,
                                         bias=bv_sb[:mw, m:m + 1],
                                         scale=1.0 / W8SCALE)

                # x_att = tanh(x_v * x_q[b]): one bcast DVE mul + one ACT tanh
                xa_mul = pl.tile([128, NM * S2], BF16, tag="xam", bufs=2)
                in0 = xv_all[:].rearrange("p (m b s) -> p m b s", m=NM, b=2)
                in1 = (xqT[:].rearrange("p (m b) -> p m b", m=NM)
                       [:, :, b0:b0 + 2].unsqueeze(3)
                       .broadcast_to([128, NM, 2, S]))
                nc.vector.tensor_mul(
                    xa_mul[:].rearrange("p (m b s) -> p m b s", m=NM, b=2),
                    in0, in1)
                xa_all = pl.tile([128, NM * S2], BF16, tag="xa", bufs=2)
                nc.scalar.activation(xa_all[:], xa_mul[:], AF.Tanh)

                # scores accumulation over DA tiles
                p_sc = pl_ps.tile([G, S2], F32, tag="psc", bufs=2)
                for m, (m0, mw) in enumerate(DA_TILES):
                    nc.tensor.matmul(p_sc[:], wa_sb[:mw, m * G:(m + 1) * G],
                                     xa_all[:mw, m * S2:(m + 1) * S2],
                                     start=(m == 0), stop=(m == NM - 1),
                                     skip_group_check=True)

                # att = exp(scores + ba) with per-sample row sums
                e_sb = pl.tile([G, S2], BF16, tag="e", bufs=2)
                esum = pl.tile([G, 2], F32, tag="esum", bufs=2)
                for s in range(2):
                    nc.scalar.activation(e_sb[:, s * S:(s + 1) * S],
                                         p_sc[:, s * S:(s + 1) * S], AF.Exp,
                                         bias=ba_sb[:], accum_out=esum[:, s:s + 1])
                recip = pl.tile([G, 2], F32, tag="recip", bufs=2)
                nc.vector.reciprocal(recip[:], esum[:])

                # e^T: 4 transposes packed into one PSUM bank + one DVE copy
                # cols (s, half, g); first transpose zeroes the whole bank.
                peT = pl_ps.tile([128, 4 * G], BF16, tag="peT", bufs=1)
                first = True
                for s in range(2):
                    nc.tensor.matmul(peT[:, s * 2 * G:s * 2 * G + G],
                                     e_sb[:, s * S:s * S + 128],
                                     ident[:G, :G], is_transpose=True,
                                     start=first, stop=False,
                                     skip_group_check=True)
                    first = False
                    nc.tensor.matmul(peT[:68, s * 2 * G + G:s * 2 * G + 2 * G],
                                     e_sb[:, s * S + 128:(s + 1) * S],
                                     ident[:G, :G], is_transpose=True,
                                     start=False, stop=(s == 1),
                                     skip_group_check=True)
                eT = pl.tile([128, 4 * G], BF16, tag="eT", bufs=2)
                nc.vector.tensor_copy(eT[:], peT[:])

                # pooling: U[g, c] = e^T @ v^T; normalize into tmp; 1 DMA to
                # vatt (DVE writes need 32-aligned partition base, DMA not)
                for s in range(2):
                    tmp = pl.tile([G, DV], BF16, tag="ptmp", bufs=2)
                    for c in range(DV // 512):
                        c0 = c * 512
                        pp = pl_ps.tile([G, 512], F32, tag="ppool", bufs=2)
                        nc.tensor.matmul(pp[:], eT[:, s * 2 * G:s * 2 * G + G],
                                         vt0[s][:, c0:c0 + 512],
                                         start=True, stop=False,
                                         skip_group_check=True)
                        nc.tensor.matmul(pp[:],
                                         eT[:68, s * 2 * G + G:s * 2 * G + 2 * G],
                                         vt1[s][:, c0:c0 + 512],
                                         start=False, stop=True,
                                         skip_group_check=True)
                        nc.vector.tensor_scalar_mul(
                            tmp[:, c0:c0 + 512], pp[:], recip[:, s:s + 1])
                    nc.sync.dma_start(
                        vatt_sb[(b0 + s) * G:(b0 + s + 1) * G, :], tmp[:])

                xqf_chunk(pair)
                # prefetch Wf during late pairs (consumed at tail start)
                if pair >= 4:
                    g2 = pair - 4
                    for kk in range(NKM):
                        load_wf(g2, kk)

        # ---------------- tail: vaT transpose, fusion, classifier ----------
        with tc.tile_pool(name="tl", bufs=1) as tl, \
             tc.tile_pool(name="tl_ps", bufs=1, space="PSUM") as tl_ps:
            bf_sb = tl.tile([1, DH], BF16)
            nc.sync.dma_start(bf_sb[:], bfb[:])
            bc_sb = tl.tile([1, NANS], BF16)
            nc.sync.dma_start(bc_sb[:], bc[:])

            # start streaming Wc immediately (classifier consumes later)
            wc_tiles = {}
            for j, (n0, nw) in enumerate(NANS_TILES):
                for kk in range(NKM):
                    t = tl.tile([128, KM * nw], BF16, tag=f"wc{j}_{kk}",
                                bufs=1)
                    nc.sync.dma_start(
                        t[:].rearrange("p (k n) -> p k n", k=KM),
                        wc[kk * KM * 128:(kk + 1) * KM * 128, n0:n0 + nw]
                        .rearrange("(k c) n -> c k n", k=KM))
                    wc_tiles[(j, kk)] = t

            # vaT[k]: [128, (b, g)] bf16; fusion uses strided per-g columns
            vaT = []
            for k in range(NK):
                p = tl_ps.tile([128, G * BPC], BF16, tag="pvat", bufs=2)
                nc.tensor.transpose(p[:], vatt_sb[:, k * 128:(k + 1) * 128],
                                    ident[:G * BPC, :G * BPC])
                t = tl.tile([128, G * BPC], BF16, tag=f"vaT{k}")
                nc.vector.tensor_copy(t[:], p[:])
                vaT.append(t)

            xv_sb = tl.tile([BPC, DH], BF16)
            out_sb = tl.tile([BPC, NANS], F32)
            for g in range(G):
                pd = tl_ps.tile([BPC, DHG], F32, tag="pd", bufs=2)
                for k in range(NK):
                    lhs = (vaT[k][:].rearrange("p (b g) -> p b g", g=G)
                           [:, :, g])
                    nc.tensor.matmul(
                        pd[:], lhs,
                        wf_tiles[(g, k // KM)][:, (k % KM) * DHG:
                                               (k % KM + 1) * DHG],
                        start=(k == 0), stop=False, skip_group_check=True)
                nc.tensor.matmul(pd[:], ones[:],
                                 bf_sb[:, g * DHG:(g + 1) * DHG],
                                 start=False, stop=True, skip_group_check=True)
                nc.scalar.activation(xv_sb[:, g * DHG:(g + 1) * DHG], pd[:],
                                     AF.Tanh)
            # x = tanh(xv * xqf) transposed into xT[k] tiles (bf16)
            xT = []
            for k in range(NK):
                xmk = tl.tile([BPC, 128], BF16, tag="xmk", bufs=3)
                nc.vector.tensor_mul(xmk[:], xv_sb[:, k * 128:(k + 1) * 128],
                                     xqf_sb[:, k * 128:(k + 1) * 128])
                px = tl_ps.tile([128, BPC], BF16, tag="pxT", bufs=2)
                nc.tensor.transpose(px[:], xmk[:], ident[:BPC, :BPC])
                xTk = tl.tile([128, BPC], BF16, tag=f"xT{k}")
                nc.scalar.activation(xTk[:], px[:], AF.Tanh)
                xT.append(xTk)
            # classifier j-outer over prefetched Wc tiles
            for j, (n0, nw) in enumerate(NANS_TILES):
                pc = tl_ps.tile([BPC, nw], F32, tag="pc", bufs=2)
                for kk in range(NKM):
                    wct = wc_tiles[(j, kk)]
                    for ki in range(KM):
                        k = kk * KM + ki
                        nc.tensor.matmul(pc[:], xT[k][:],
                                         wct[:, ki * nw:(ki + 1) * nw],
                                         start=(k == 0), stop=False,
                                         skip_group_check=True)
                nc.tensor.matmul(pc[:], ones[:], bc_sb[:, n0:n0 + nw],
                                 start=False, stop=True, skip_group_check=True)
                nc.vector.tensor_copy(out_sb[:, n0:n0 + nw], pc[:])
            nc.sync.dma_start(out[:], out_sb[:])


_module_cache = {}


def _get_module(reps: int = 1):
    if reps not in _module_cache:
        _module_cache[reps] = build_module(reps)
    return _module_cache[reps]


def make_in_maps(inputs: dict) -> list:
    F8NP = ml_dtypes.float8_e4m3
    BFNP = ml_dtypes.bfloat16
    iv = np.ascontiguousarray(inputs["input_v"], np.float32).reshape(B, DV, S)
    xq = np.ascontiguousarray(inputs["x_q_vec"], np.float32)

    # Wv: scale, quantize fp8, pack [NKK, 128, (ki, d)]
    wv_s = (np.asarray(inputs["Wv_att"], np.float32) * W8SCALE).astype(F8NP)
    wv_pk = np.ascontiguousarray(
        wv_s.reshape(NKK, KM8, 128, DA).transpose(0, 2, 1, 3)
    ).reshape(NKK, 128, KM8 * DA)

    shared = {
        "wv8": wv_pk,
        "bv": np.ascontiguousarray(inputs["bv_att"], np.float32).reshape(DA, 1),
        "wq": np.asarray(inputs["Wq_att"]).astype(BFNP),
        "bq": np.ascontiguousarray(inputs["bq_att"], np.float32).reshape(DA, 1),
        "wa": np.asarray(inputs["Wa"]).astype(BFNP),
        "ba": np.ascontiguousarray(inputs["ba"], np.float32).reshape(G, 1),
        "wf": np.asarray(inputs["Wf"]).astype(BFNP),
        "bfb": np.asarray(inputs["bf"]).astype(BFNP).reshape(1, DH),
        "wqf": np.asarray(inputs["Wqf"]).astype(BFNP),
        "bqf": np.asarray(inputs["bqf"]).astype(BFNP).reshape(1, DH),
        "wc": np.asarray(inputs["Wc"]).astype(BFNP),
        "bc": np.asarray(inputs["bc"]).astype(BFNP).reshape(1, NANS),
    }
    in_maps = []
    for c in range(NCORES):
        vv = iv[c * BPC:(c + 1) * BPC]                       # [BPC, DV, S]
        v8c = vv.astype(F8NP)
        # [NPAIR, NKK, 128, (ki, b, s)]
        v8p = np.ascontiguousarray(
            v8c.reshape(NPAIR, 2, NKK, KM8, 128, S).transpose(0, 2, 4, 3, 1, 5)
        ).reshape(NPAIR, NKK, 128, KM8 * 2 * S)
        vtc = np.ascontiguousarray(vv.transpose(0, 2, 1)).astype(BFNP)
        m = dict(shared)
        m["v8"] = v8p
        m["vt"] = vtc
        m["q"] = xq[c * BPC:(c + 1) * BPC].astype(BFNP)
        in_maps.append(m)
    return in_maps


def kernel(**inputs) -> np.ndarray:
    nc = _get_module(1)
    in_maps = make_in_maps(inputs)
    res = run_bass_kernel_spmd(nc, in_maps, core_ids=list(range(NCORES)))
    return np.concatenate([res.results[c]["out"] for c in range(NCORES)], axis=0)
